# revision 6
# baseline (speedup 1.0000x reference)
"""MlssaSelector Trainium2 kernel (8-core SPMD, data-parallel over bags).

v3: single-layout fp8 shipping + fp8 DoubleRow matmuls + y-restructure.

x is shipped ONCE, transposed, as an fp8(e4m3) pair  a8 = q8(xT),
b8 = q8(xT - a8)  (the same-scale residual lands in e4m3 denormals and
still cuts x quantization error ~30x).  HBM traffic per core: 16.8MB,
half the old two-copy bf16 scheme.  Weights are pre-scaled by 64 on the
host so their fp8 residuals are normal-range; the 1/64 folds into the
ACT tanh scale (scores path) and into the ones-matmul constant (y path).

Per core (16384 sentences = 1024 bags, boundaries shard-aligned):
  sT  = (a8+b8)^T W1s8 + a8^T W1sr8   (PE, 6 fp8-DR matmuls / 512-supertile
                                       per a-half; DR measured ~5x bf16)
  th  = tanh(sT / 64)                 (ACT, PSUM->SBUF bf16)
  zT  = W2^T th                       (PE, bf16)
  e   = exp(zT)                       (ACT; |z|<~3 so no max-subtraction)
  den = segsum_16(e); rden = 1/den    (DVE grouped reduce + reciprocal)
  emul= e * rden[bag]                 (DVE, free-broadcast of rden)
  wB  = (0.25/64)*ones53^T emul       (PE: head-sum + broadcast to 53 parts)
  yT  = Wcs8^T(a8+b8) + Wcsr8^T a8    (PE, 6 fp8-DR matmuls; y = 64*x@Wc)
  yw  = yT * wB                       (DVE, dual-PSUM read)
  logT= segsum_16(yw)                 (DVE grouped reduce -> [53, bags])
  out = (logT + bc)^T via PE transpose, f32 DMA out per 128-bag group
"""

import numpy as np
import ml_dtypes

import concourse.bacc as bacc
import concourse.mybir as mybir
import concourse.tile as tile
from concourse import bass_utils

BF16 = ml_dtypes.bfloat16
F8 = ml_dtypes.float8_e4m3

N = 131072
D = 512
A = 256          # D_ATT
H = 4            # heads
C = 53           # classes
CP = 64          # C padded for fp8-DR ldweights (dual-fp8 wants 64-mult cols)
BAG = 16
NCORES = 8
NSH = N // NCORES          # 16384 sentences per core
BSH = NSH // BAG           # 1024 bags per core
NT = 512                   # sentences per supertile
NSUP = NSH // NT           # 32 supertiles
BAGS_PER_SUP = NT // BAG   # 32
WS = 64.0                  # host-side weight pre-scale

_CACHE = {}


def _build_nc(repeat=1, skip=(), bufs=None, pipe=3, p2off=4):
    nc = bacc.Bacc("TRN2", target_bir_lowering=False, debug=False)
    dt = mybir.dt
    DR = mybir.MatmulPerfMode.DoubleRow

    a8_d = nc.dram_tensor("a8", [D, NSH], dt.float8e4, kind="ExternalInput")
    b8_d = nc.dram_tensor("b8", [D, NSH], dt.float8e4, kind="ExternalInput")
    w1a_d = nc.dram_tensor("w1a", [D, A], dt.float8e4, kind="ExternalInput")
    w1b_d = nc.dram_tensor("w1b", [D, A], dt.float8e4, kind="ExternalInput")
    w2_d = nc.dram_tensor("w2", [A, H], dt.bfloat16, kind="ExternalInput")
    wca_d = nc.dram_tensor("wca", [D, CP], dt.float8e4, kind="ExternalInput")
    wcb_d = nc.dram_tensor("wcb", [D, CP], dt.float8e4, kind="ExternalInput")
    ones_d = nc.dram_tensor("ones53", [H, CP], dt.bfloat16, kind="ExternalInput")
    id_d = nc.dram_tensor("id53", [CP, CP], dt.float32, kind="ExternalInput")
    bc_d = nc.dram_tensor("bc53", [CP, 1], dt.float32, kind="ExternalInput")
    out_d = nc.dram_tensor("logits", [BSH, C], dt.float32, kind="ExternalOutput")

    a8_r = a8_d[:].rearrange("(dc p) n -> p dc n", p=128)   # [128,4,NSH]
    b8_r = b8_d[:].rearrange("(dc p) n -> p dc n", p=128)
    out_r = out_d[:].rearrange("(t p) c -> p t c", p=128)   # [128,8,53]

    B = dict(xin=11, th=7, small=7, s=2, z=2, y=2, w=1, t=1)
    if bufs:
        B.update(bufs)
    with tile.TileContext(nc) as tc:
        with (
            tc.tile_pool(name="consts", bufs=1) as consts,
            tc.tile_pool(name="xin", bufs=B["xin"]) as xin,
            tc.tile_pool(name="thp", bufs=B["th"]) as thp,
            tc.tile_pool(name="small", bufs=B["small"]) as small,
            tc.tile_pool(name="persist", bufs=1) as persist,
            tc.tile_pool(name="ps_s", bufs=B["s"], space="PSUM") as ps_s_pool,
            tc.tile_pool(name="ps_z", bufs=B["z"], space="PSUM") as ps_z_pool,
            tc.tile_pool(name="ps_y", bufs=B["y"], space="PSUM") as ps_y_pool,
            tc.tile_pool(name="ps_w", bufs=B["w"], space="PSUM") as ps_w_pool,
            tc.tile_pool(name="ps_t", bufs=B["t"], space="PSUM") as ps_t_pool,
        ):
            # ---- constants via gpsimd SWDGE (keeps HWDGE free for x) ----
            w1a_sb = consts.tile([128, 4, A], dt.float8e4)
            nc.gpsimd.dma_start(out=w1a_sb, in_=w1a_d[:].rearrange("(dc p) a -> p dc a", p=128))
            w1b_sb = consts.tile([128, 4, A], dt.float8e4)
            nc.gpsimd.dma_start(out=w1b_sb, in_=w1b_d[:].rearrange("(dc p) a -> p dc a", p=128))
            w2_sb = consts.tile([128, 2, H], dt.bfloat16)
            nc.gpsimd.dma_start(out=w2_sb, in_=w2_d[:].rearrange("(ah p) h -> p ah h", p=128))
            wca_sb = consts.tile([128, 4, CP], dt.float8e4)
            nc.gpsimd.dma_start(out=wca_sb, in_=wca_d[:].rearrange("(dc p) c -> p dc c", p=128))
            wcb_sb = consts.tile([128, 4, CP], dt.float8e4)
            nc.gpsimd.dma_start(out=wcb_sb, in_=wcb_d[:].rearrange("(dc p) c -> p dc c", p=128))
            ones_sb = consts.tile([H, CP], dt.bfloat16)
            nc.gpsimd.dma_start(out=ones_sb, in_=ones_d[:])
            id_sb = consts.tile([CP, CP], dt.float32)
            nc.gpsimd.dma_start(out=id_sb, in_=id_d[:])
            bc_sb = consts.tile([CP, 1], dt.float32)
            nc.gpsimd.dma_start(out=bc_sb, in_=bc_d[:])

            for _rep in range(repeat):
              logT = persist.tile([CP, BSH], dt.float32, tag="logT")
              log_out = persist.tile([128, BSH // 128, CP], dt.float32, tag="lo")
              state = {}
              state2 = {}

              def emit_front(j):
                  a8_t = xin.tile([128, 4, NT], dt.float8e4, tag="a8")
                  nc.sync.dma_start(out=a8_t, in_=a8_r[:, :, j * NT:(j + 1) * NT])
                  b8_t = xin.tile([128, 4, NT], dt.float8e4, tag="b8")
                  nc.gpsimd.dma_start(out=b8_t, in_=b8_r[:, :, j * NT:(j + 1) * NT])
                  state[j] = (a8_t, b8_t)

              def emit_mid(j):
                  a8_t, b8_t = state.pop(j)
                  # sT = (a8+b8)@W1s8 + a8@W1sr8, 6 fp8-DR matmuls per a-half
                  th_t = thp.tile([128, 2, NT], dt.bfloat16, tag="th")
                  terms = [(a8_t, w1a_sb), (b8_t, w1a_sb), (a8_t, w1b_sb)]
                  for ah in range(2):
                      ps_s = ps_s_pool.tile([128, NT], dt.float32, tag="s")
                      if "s1" not in skip:
                          for t, (xt, wt) in enumerate(terms):
                              for c in range(2):
                                  nc.tensor.matmul(
                                      ps_s,
                                      wt[:, 2 * c:2 * c + 2, ah * 128:(ah + 1) * 128],
                                      xt[:, 2 * c:2 * c + 2, :],
                                      start=(t == 0 and c == 0),
                                      stop=(t == 2 and c == 1),
                                      perf_mode=DR,
                                  )
                      if "tanh" not in skip:
                          nc.scalar.activation(
                              th_t[:, ah, :], ps_s,
                              mybir.ActivationFunctionType.Tanh, scale=1.0 / WS,
                          )
                  state2[j] = (a8_t, b8_t, th_t)

              def emit_back(j):
                  a8_t, b8_t, th_t = state2.pop(j)
                  # zT = W2.T @ th  -> [4, NT]
                  ps_z = ps_z_pool.tile([H, NT], dt.float32, tag="zz")
                  if "z" not in skip:
                      for ah in range(2):
                          nc.tensor.matmul(
                              ps_z, w2_sb[:, ah, :], th_t[:, ah, :],
                              start=ah == 0, stop=ah == 1,
                          )

                  # segment softmax pieces (no max subtraction needed)
                  e_t = small.tile([H, NT], dt.bfloat16, tag="e")
                  nc.scalar.activation(e_t, ps_z, mybir.ActivationFunctionType.Exp)
                  den = small.tile([H, BAGS_PER_SUP], dt.float32, tag="den")
                  nc.vector.tensor_reduce(
                      den, e_t.rearrange("h (b i) -> h b i", i=BAG),
                      axis=mybir.AxisListType.X, op=mybir.AluOpType.add,
                  )
                  rdbf = small.tile([H, BAGS_PER_SUP], dt.bfloat16, tag="rd")
                  with nc.allow_low_precision("rden feeds bf16 emul anyway"):
                      nc.vector.reciprocal(rdbf, den)
                  emul = small.tile([H, NT], dt.bfloat16, tag="em")
                  nc.vector.tensor_tensor(
                      out=emul.rearrange("h (b i) -> h b i", i=BAG),
                      in0=e_t.rearrange("h (b i) -> h b i", i=BAG),
                      in1=rdbf[:, :, None].broadcast_to([H, BAGS_PER_SUP, BAG]),
                      op=mybir.AluOpType.mult,
                  )

                  # wB[c, n] = (0.25/64) * sum_h emul[h, n]  (head-sum + bcast)
                  ps_w = ps_w_pool.tile([CP, NT], dt.float32, tag="w")
                  if "w" not in skip:
                      nc.tensor.matmul(ps_w, ones_sb, emul, start=True, stop=True)

                  # yT = Wcs8.T(a8+b8) + Wcsr8.T a8, 6 fp8-DR matmuls
                  ps_y = ps_y_pool.tile([CP, NT], dt.float32, tag="y")
                  if "y" not in skip:
                      terms = [(a8_t, wca_sb), (b8_t, wca_sb), (a8_t, wcb_sb)]
                      for t, (xt, wt) in enumerate(terms):
                          for c in range(2):
                              nc.tensor.matmul(
                                  ps_y,
                                  wt[:, 2 * c:2 * c + 2, :],
                                  xt[:, 2 * c:2 * c + 2, :],
                                  start=(t == 0 and c == 0),
                                  stop=(t == 2 and c == 1),
                                  perf_mode=DR,
                              )

                  # walrus: only one non-scalar input may live in PSUM, so
                  # drain wB to SBUF on the (idle-ish) scalar engine first
                  wb_sb = small.tile([CP, NT], dt.bfloat16, tag="wb")
                  nc.scalar.activation(wb_sb, ps_w,
                                       mybir.ActivationFunctionType.Copy)
                  yw = small.tile([CP, NT], dt.bfloat16, tag="yw")
                  nc.vector.tensor_tensor(
                      out=yw, in0=ps_y, in1=wb_sb, op=mybir.AluOpType.mult,
                  )
                  if "red" not in skip:
                      nc.vector.tensor_reduce(
                          logT[:, j * BAGS_PER_SUP:(j + 1) * BAGS_PER_SUP],
                          yw.rearrange("c (b i) -> c b i", i=BAG),
                          axis=mybir.AxisListType.X, op=mybir.AluOpType.add,
                      )

                  # tail per 128-bag group: bias, transpose, store
                  if j % 4 == 3:
                      g = j // 4
                      tmpb = small.tile([CP, 128], dt.float32, tag="tb")
                      nc.vector.tensor_scalar(
                          out=tmpb, in0=logT[:, g * 128:(g + 1) * 128],
                          scalar1=bc_sb, scalar2=None, op0=mybir.AluOpType.add,
                      )
                      ps_t = ps_t_pool.tile([128, CP], dt.float32, tag="t")
                      nc.tensor.matmul(ps_t, tmpb, id_sb, start=True, stop=True,
                                       is_transpose=True)
                      nc.any.tensor_copy(log_out[:, g, :], ps_t)
                      oeng = nc.sync if g == BSH // 128 - 1 else nc.gpsimd
                      oeng.dma_start(out=out_r[:, g, :], in_=log_out[:, g, 0:C])

              P1 = pipe
              P2 = pipe + p2off
              for j in range(NSUP + P2):
                  if j < NSUP:
                      emit_front(j)
                  if P1 <= j < NSUP + P1:
                      emit_mid(j - P1)
                  if j >= P2:
                      emit_back(j - P2)

    nc.compile()
    return nc


def _get_nc():
    if "nc" not in _CACHE:
        _CACHE["nc"] = _build_nc()
    return _CACHE["nc"]


def _host_prep(x, W1, W2, Wc, bc, seg_ids, n_bags):
    x = np.asarray(x, dtype=np.float32)
    W1 = np.asarray(W1, dtype=np.float32)
    W2 = np.asarray(W2, dtype=np.float32)
    Wc = np.asarray(Wc, dtype=np.float32)
    bc = np.asarray(bc, dtype=np.float32)

    w1s = WS * W1
    w1a = w1s.astype(F8)
    w1b = (w1s - w1a.astype(np.float32)).astype(F8)
    wc_pad = np.zeros((D, CP), np.float32)
    wc_pad[:, :C] = Wc
    wcs = WS * wc_pad
    wca = wcs.astype(F8)
    wcb = (wcs - wca.astype(np.float32)).astype(F8)
    w2b = W2.astype(BF16)
    ones53 = np.full((H, CP), 0.25 / WS, np.float32).astype(BF16)
    id53 = np.eye(CP, dtype=np.float32)
    bc53 = np.zeros((CP, 1), np.float32)
    bc53[:C, 0] = bc

    in_maps = []
    for c in range(NCORES):
        xT = np.ascontiguousarray(x[c * NSH:(c + 1) * NSH].T)
        a8 = xT.astype(F8)
        b8 = (xT - a8.astype(np.float32)).astype(F8)
        in_maps.append({
            "a8": a8, "b8": b8,
            "w1a": w1a, "w1b": w1b, "w2": w2b,
            "wca": wca, "wcb": wcb,
            "ones53": ones53, "id53": id53, "bc53": bc53,
        })
    return in_maps


def kernel(x, W1, W2, Wc, bc, seg_ids, n_bags, _trace=False):
    in_maps = _host_prep(x, W1, W2, Wc, bc, seg_ids, n_bags)
    nc = _get_nc()
    res = bass_utils.run_bass_kernel_spmd(
        nc, in_maps, core_ids=list(range(NCORES)), trace=_trace,
    )
    out = np.concatenate([r["logits"] for r in res.results], axis=0)
    if _trace:
        kernel.last_results = res
    return out


# revision 8
# speedup vs baseline: 1.1611x; 1.1611x over previous
"""MlssaSelector Trainium2 kernel (8-core SPMD, data-parallel over bags).

v3: single-layout fp8 shipping + fp8 DoubleRow matmuls + y-restructure.

x is shipped ONCE, transposed, as an fp8(e4m3) pair  a8 = q8(xT),
b8 = q8(xT - a8)  (the same-scale residual lands in e4m3 denormals and
still cuts x quantization error ~30x).  HBM traffic per core: 16.8MB,
half the old two-copy bf16 scheme.  Weights are pre-scaled by 64 on the
host so their fp8 residuals are normal-range; the 1/64 folds into the
ACT tanh scale (scores path) and into the ones-matmul constant (y path).

Per core (16384 sentences = 1024 bags, boundaries shard-aligned):
  sT  = (a8+b8)^T W1s8 + a8^T W1sr8   (PE, 6 fp8-DR matmuls / 512-supertile
                                       per a-half; DR measured ~5x bf16)
  th  = tanh(sT / 64)                 (ACT, PSUM->SBUF bf16)
  zT  = W2^T th                       (PE, bf16)
  e   = exp(zT)                       (ACT; |z|<~3 so no max-subtraction)
  den = segsum_16(e); rden = 1/den    (DVE grouped reduce + reciprocal)
  emul= e * rden[bag]                 (DVE, free-broadcast of rden)
  wB  = (0.25/64)*ones53^T emul       (PE: head-sum + broadcast to 53 parts)
  yT  = Wcs8^T(a8+b8) + Wcsr8^T a8    (PE, 6 fp8-DR matmuls; y = 64*x@Wc)
  yw  = yT * wB                       (DVE, dual-PSUM read)
  logT= segsum_16(yw)                 (DVE grouped reduce -> [53, bags])
  out = (logT + bc)^T via PE transpose, f32 DMA out per 128-bag group
"""

import numpy as np
import ml_dtypes

import concourse.bacc as bacc
import concourse.mybir as mybir
import concourse.tile as tile
from concourse import bass_utils

BF16 = ml_dtypes.bfloat16
F8 = ml_dtypes.float8_e4m3

N = 131072
D = 512
A = 256          # D_ATT
H = 4            # heads
C = 53           # classes
CP = 64          # C padded for fp8-DR ldweights (dual-fp8 wants 64-mult cols)
BAG = 16
NCORES = 8
NSH = N // NCORES          # 16384 sentences per core
BSH = NSH // BAG           # 1024 bags per core
NT = 512                   # sentences per supertile
NSUP = NSH // NT           # 32 supertiles
BAGS_PER_SUP = NT // BAG   # 32
WS = 64.0                  # host-side weight pre-scale

_CACHE = {}


def _build_nc(repeat=1, skip=(), bufs=None, pipe=3, p2off=4):
    nc = bacc.Bacc("TRN2", target_bir_lowering=False, debug=False)
    dt = mybir.dt
    DR = mybir.MatmulPerfMode.DoubleRow

    a8_d = nc.dram_tensor("a8", [D, NSH], dt.float8e4, kind="ExternalInput")
    b8_d = nc.dram_tensor("b8", [D, NSH], dt.float8e4, kind="ExternalInput")
    w1a_d = nc.dram_tensor("w1a", [D, A], dt.float8e4, kind="ExternalInput")
    w1b_d = nc.dram_tensor("w1b", [D, A], dt.float8e4, kind="ExternalInput")
    w2_d = nc.dram_tensor("w2", [A, H], dt.bfloat16, kind="ExternalInput")
    wca_d = nc.dram_tensor("wca", [D, CP], dt.float8e4, kind="ExternalInput")
    wcb_d = nc.dram_tensor("wcb", [D, CP], dt.float8e4, kind="ExternalInput")
    ones_d = nc.dram_tensor("ones53", [H, CP], dt.bfloat16, kind="ExternalInput")
    id_d = nc.dram_tensor("id53", [CP, CP], dt.float32, kind="ExternalInput")
    bc_d = nc.dram_tensor("bc53", [CP, 1], dt.float32, kind="ExternalInput")
    out_d = nc.dram_tensor("logits", [BSH, C], dt.float32, kind="ExternalOutput")

    a8_r = a8_d[:].rearrange("(dc p) n -> p dc n", p=128)   # [128,4,NSH]
    b8_r = b8_d[:].rearrange("(dc p) n -> p dc n", p=128)
    out_r = out_d[:].rearrange("(t p) c -> p t c", p=128)   # [128,8,53]

    B = dict(xin=11, th=7, small=7, s=2, z=2, y=2, w=1, t=1)
    if bufs:
        B.update(bufs)
    with tile.TileContext(nc) as tc:
        with (
            tc.tile_pool(name="consts", bufs=1) as consts,
            tc.tile_pool(name="xin", bufs=B["xin"]) as xin,
            tc.tile_pool(name="thp", bufs=B["th"]) as thp,
            tc.tile_pool(name="small", bufs=B["small"]) as small,
            tc.tile_pool(name="persist", bufs=1) as persist,
            tc.tile_pool(name="ps_s", bufs=B["s"], space="PSUM") as ps_s_pool,
            tc.tile_pool(name="ps_z", bufs=B["z"], space="PSUM") as ps_z_pool,
            tc.tile_pool(name="ps_y", bufs=B["y"], space="PSUM") as ps_y_pool,
            tc.tile_pool(name="ps_w", bufs=B["w"], space="PSUM") as ps_w_pool,
            tc.tile_pool(name="ps_t", bufs=B["t"], space="PSUM") as ps_t_pool,
        ):
            # ---- constants via gpsimd SWDGE (keeps HWDGE free for x) ----
            w1a_sb = consts.tile([128, 4, A], dt.float8e4)
            nc.gpsimd.dma_start(out=w1a_sb, in_=w1a_d[:].rearrange("(dc p) a -> p dc a", p=128))
            w1b_sb = consts.tile([128, 4, A], dt.float8e4)
            nc.gpsimd.dma_start(out=w1b_sb, in_=w1b_d[:].rearrange("(dc p) a -> p dc a", p=128))
            w2_sb = consts.tile([128, 2, H], dt.bfloat16)
            nc.gpsimd.dma_start(out=w2_sb, in_=w2_d[:].rearrange("(ah p) h -> p ah h", p=128))
            wca_sb = consts.tile([128, 4, CP], dt.float8e4)
            nc.gpsimd.dma_start(out=wca_sb, in_=wca_d[:].rearrange("(dc p) c -> p dc c", p=128))
            wcb_sb = consts.tile([128, 4, CP], dt.float8e4)
            nc.gpsimd.dma_start(out=wcb_sb, in_=wcb_d[:].rearrange("(dc p) c -> p dc c", p=128))
            ones_sb = consts.tile([H, CP], dt.bfloat16)
            nc.gpsimd.dma_start(out=ones_sb, in_=ones_d[:])
            id_sb = consts.tile([CP, CP], dt.float32)
            nc.gpsimd.dma_start(out=id_sb, in_=id_d[:])
            bc_sb = consts.tile([CP, 1], dt.float32)
            nc.gpsimd.dma_start(out=bc_sb, in_=bc_d[:])

            for _rep in range(repeat):
              logT = persist.tile([CP, BSH], dt.float32, tag="logT")
              log_out = persist.tile([128, BSH // 128, CP], dt.float32, tag="lo")
              state = {}
              state2 = {}
              state3 = {}

              def emit_front(j):
                  a8_t = xin.tile([128, 4, NT], dt.float8e4, tag="a8")
                  nc.sync.dma_start(out=a8_t, in_=a8_r[:, :, j * NT:(j + 1) * NT])
                  b8_t = xin.tile([128, 4, NT], dt.float8e4, tag="b8")
                  nc.gpsimd.dma_start(out=b8_t, in_=b8_r[:, :, j * NT:(j + 1) * NT])
                  state[j] = (a8_t, b8_t)

              def emit_mid(j):
                  a8_t, b8_t = state.pop(j)
                  # sT = (a8+b8)@W1s8 + a8@W1sr8, 6 fp8-DR matmuls per a-half
                  th_t = thp.tile([128, 2, NT], dt.bfloat16, tag="th")
                  terms = [(a8_t, w1a_sb), (b8_t, w1a_sb), (a8_t, w1b_sb)]
                  for ah in range(2):
                      ps_s = ps_s_pool.tile([128, NT], dt.float32, tag="s")
                      if "s1" not in skip:
                          for t, (xt, wt) in enumerate(terms):
                              for c in range(2):
                                  nc.tensor.matmul(
                                      ps_s,
                                      wt[:, 2 * c:2 * c + 2, ah * 128:(ah + 1) * 128],
                                      xt[:, 2 * c:2 * c + 2, :],
                                      start=(t == 0 and c == 0),
                                      stop=(t == 2 and c == 1),
                                      perf_mode=DR,
                                  )
                      if "tanh" not in skip:
                          nc.scalar.activation(
                              th_t[:, ah, :], ps_s,
                              mybir.ActivationFunctionType.Tanh, scale=1.0 / WS,
                          )
                  state2[j] = (a8_t, b8_t, th_t)

              def emit_zst(j):
                  a8_t, b8_t, th_t = state2.pop(j)
                  # zT = W2.T @ th  -> [4, NT]
                  ps_z = ps_z_pool.tile([H, NT], dt.float32, tag="zz")
                  if "z" not in skip:
                      for ah in range(2):
                          nc.tensor.matmul(
                              ps_z, w2_sb[:, ah, :], th_t[:, ah, :],
                              start=ah == 0, stop=ah == 1,
                          )

                  # segment softmax pieces (no max subtraction needed)
                  e_t = small.tile([H, NT], dt.bfloat16, tag="e")
                  nc.scalar.activation(e_t, ps_z, mybir.ActivationFunctionType.Exp)
                  den = small.tile([H, BAGS_PER_SUP], dt.float32, tag="den")
                  nc.vector.tensor_reduce(
                      den, e_t.rearrange("h (b i) -> h b i", i=BAG),
                      axis=mybir.AxisListType.X, op=mybir.AluOpType.add,
                  )
                  rdbf = small.tile([H, BAGS_PER_SUP], dt.bfloat16, tag="rd")
                  with nc.allow_low_precision("rden feeds bf16 emul anyway"):
                      nc.vector.reciprocal(rdbf, den)
                  emul = small.tile([H, NT], dt.bfloat16, tag="em")
                  nc.vector.tensor_tensor(
                      out=emul.rearrange("h (b i) -> h b i", i=BAG),
                      in0=e_t.rearrange("h (b i) -> h b i", i=BAG),
                      in1=rdbf[:, :, None].broadcast_to([H, BAGS_PER_SUP, BAG]),
                      op=mybir.AluOpType.mult,
                  )
                  state3[j] = (a8_t, b8_t, emul)

              def emit_yst(j):
                  a8_t, b8_t, emul = state3.pop(j)
                  # wB[c, n] = (0.25/64) * sum_h emul[h, n]  (head-sum + bcast)
                  # emul was computed 2 supertiles ago, so the in-order PE
                  # queue does not stall on the DVE here
                  ps_w = ps_w_pool.tile([CP, NT], dt.float32, tag="w")
                  if "w" not in skip:
                      nc.tensor.matmul(ps_w, ones_sb, emul, start=True, stop=True)
                  # walrus: only one non-scalar input may live in PSUM, so
                  # drain wB to SBUF on the scalar engine while PE runs y
                  wb_sb = small.tile([CP, NT], dt.bfloat16, tag="wb")
                  nc.scalar.activation(wb_sb, ps_w,
                                       mybir.ActivationFunctionType.Copy)

                  # yT = Wcs8.T(a8+b8) + Wcsr8.T a8, 6 fp8-DR matmuls
                  ps_y = ps_y_pool.tile([CP, NT], dt.float32, tag="y")
                  if "y" not in skip:
                      terms = [(a8_t, wca_sb), (b8_t, wca_sb), (a8_t, wcb_sb)]
                      for t, (xt, wt) in enumerate(terms):
                          for c in range(2):
                              nc.tensor.matmul(
                                  ps_y,
                                  wt[:, 2 * c:2 * c + 2, :],
                                  xt[:, 2 * c:2 * c + 2, :],
                                  start=(t == 0 and c == 0),
                                  stop=(t == 2 and c == 1),
                                  perf_mode=DR,
                              )

                  yw = small.tile([CP, NT], dt.bfloat16, tag="yw")
                  nc.vector.tensor_tensor(
                      out=yw, in0=ps_y, in1=wb_sb, op=mybir.AluOpType.mult,
                  )
                  if "red" not in skip:
                      nc.vector.tensor_reduce(
                          logT[:, j * BAGS_PER_SUP:(j + 1) * BAGS_PER_SUP],
                          yw.rearrange("c (b i) -> c b i", i=BAG),
                          axis=mybir.AxisListType.X, op=mybir.AluOpType.add,
                      )

              def emit_tail(j):
                  # per 128-bag group: bias, transpose, store; runs 2 supertiles
                  # after the group's last reduce so PE never waits on the DVE
                  if j % 4 != 3:
                      return
                  g = j // 4
                  tmpb = small.tile([CP, 128], dt.float32, tag="tb")
                  nc.vector.tensor_scalar(
                      out=tmpb, in0=logT[:, g * 128:(g + 1) * 128],
                      scalar1=bc_sb, scalar2=None, op0=mybir.AluOpType.add,
                  )
                  ps_t = ps_t_pool.tile([128, CP], dt.float32, tag="t")
                  nc.tensor.matmul(ps_t, tmpb, id_sb, start=True, stop=True,
                                   is_transpose=True)
                  nc.any.tensor_copy(log_out[:, g, :], ps_t)
                  oeng = nc.sync if g == BSH // 128 - 1 else nc.gpsimd
                  oeng.dma_start(out=out_r[:, g, :], in_=log_out[:, g, 0:C])

              P1 = pipe
              P2 = P1 + p2off
              P3 = P2 + 2
              P4 = P3 + 2
              for j in range(NSUP + P4):
                  if j < NSUP:
                      emit_front(j)
                  if P1 <= j < NSUP + P1:
                      emit_mid(j - P1)
                  if P2 <= j < NSUP + P2:
                      emit_zst(j - P2)
                  if P3 <= j < NSUP + P3:
                      emit_yst(j - P3)
                  if j >= P4:
                      emit_tail(j - P4)

    nc.compile()
    return nc


def _get_nc():
    if "nc" not in _CACHE:
        _CACHE["nc"] = _build_nc()
    return _CACHE["nc"]


def _host_prep(x, W1, W2, Wc, bc, seg_ids, n_bags):
    x = np.asarray(x, dtype=np.float32)
    W1 = np.asarray(W1, dtype=np.float32)
    W2 = np.asarray(W2, dtype=np.float32)
    Wc = np.asarray(Wc, dtype=np.float32)
    bc = np.asarray(bc, dtype=np.float32)

    w1s = WS * W1
    w1a = w1s.astype(F8)
    w1b = (w1s - w1a.astype(np.float32)).astype(F8)
    wc_pad = np.zeros((D, CP), np.float32)
    wc_pad[:, :C] = Wc
    wcs = WS * wc_pad
    wca = wcs.astype(F8)
    wcb = (wcs - wca.astype(np.float32)).astype(F8)
    w2b = W2.astype(BF16)
    ones53 = np.full((H, CP), 0.25 / WS, np.float32).astype(BF16)
    id53 = np.eye(CP, dtype=np.float32)
    bc53 = np.zeros((CP, 1), np.float32)
    bc53[:C, 0] = bc

    in_maps = []
    for c in range(NCORES):
        xT = np.ascontiguousarray(x[c * NSH:(c + 1) * NSH].T)
        a8 = xT.astype(F8)
        b8 = (xT - a8.astype(np.float32)).astype(F8)
        in_maps.append({
            "a8": a8, "b8": b8,
            "w1a": w1a, "w1b": w1b, "w2": w2b,
            "wca": wca, "wcb": wcb,
            "ones53": ones53, "id53": id53, "bc53": bc53,
        })
    return in_maps


def kernel(x, W1, W2, Wc, bc, seg_ids, n_bags, _trace=False):
    in_maps = _host_prep(x, W1, W2, Wc, bc, seg_ids, n_bags)
    nc = _get_nc()
    res = bass_utils.run_bass_kernel_spmd(
        nc, in_maps, core_ids=list(range(NCORES)), trace=_trace,
    )
    out = np.concatenate([r["logits"] for r in res.results], axis=0)
    if _trace:
        kernel.last_results = res
    return out


# revision 23
# speedup vs baseline: 2.2326x; 1.9228x over previous
"""MlssaSelector Trainium2 kernel (8-core SPMD, data-parallel over bags).

v4: single-layout fp8 shipping + fp8 DoubleRow matmuls + y-restructure
+ paired z-chain: z outputs of two supertiles land in one PSUM tile at
partition offsets 0/32 (non-DR matmuls may target 32-aligned offsets;
DR and transpose matmuls must target partition 0), so exp/den/rden/emul
each run ONCE per pair -- the narrow 4-partition softmax ops were the
dominant DVE/ACT cost.  Known limits hit here: fp8-DR ldweights need
64-multiple stationary widths; lane-locked DVE/ACT cannot shift
partitions, so the y-side stays per-supertile.

x is shipped ONCE, transposed, as an fp8(e4m3) pair  a8 = q8(xT),
b8 = q8(xT - a8)  (the same-scale residual lands in e4m3 denormals and
still cuts x quantization error ~30x).  HBM traffic per core: 16.8MB,
half the old two-copy bf16 scheme.  Weights are pre-scaled by 64 on the
host so their fp8 residuals are normal-range; the 1/64 folds into the
ACT tanh scale (scores path) and into the ones-matmul constant (y path).

Per core (16384 sentences = 1024 bags, boundaries shard-aligned):
  sT  = (a8+b8)^T W1s8 + a8^T W1sr8   (PE, 6 fp8-DR matmuls / 512-supertile
                                       per a-half; DR measured ~5x bf16)
  th  = tanh(sT / 64)                 (ACT, PSUM->SBUF bf16)
  zT  = W2^T th                       (PE, bf16)
  e   = exp(zT)                       (ACT; |z|<~3 so no max-subtraction)
  den = segsum_16(e); rden = 1/den    (DVE grouped reduce + reciprocal)
  emul= e * rden[bag]                 (DVE, free-broadcast of rden)
  wB  = (0.25/64)*ones53^T emul       (PE: head-sum + broadcast to 53 parts)
  yT  = Wcs8^T(a8+b8) + Wcsr8^T a8    (PE, 6 fp8-DR matmuls; y = 64*x@Wc)
  yw  = yT * wB                       (DVE, dual-PSUM read)
  logT= segsum_16(yw)                 (DVE grouped reduce -> [53, bags])
  out = (logT + bc)^T via PE transpose, f32 DMA out per 128-bag group
"""

import numpy as np
import ml_dtypes

import concourse.bacc as bacc
import concourse.mybir as mybir
import concourse.tile as tile
from concourse import bass_utils

BF16 = ml_dtypes.bfloat16
F8 = ml_dtypes.float8_e4m3

N = 131072
D = 512
A = 256          # D_ATT
H = 4            # heads
C = 53           # classes
CP = 64          # C padded for fp8-DR ldweights (dual-fp8 wants 64-mult cols)
BAG = 16
NCORES = 8
NSH = N // NCORES          # 16384 sentences per core
BSH = NSH // BAG           # 1024 bags per core
NT = 512                   # sentences per supertile
NSUP = NSH // NT           # 32 supertiles
BAGS_PER_SUP = NT // BAG   # 32
WS = 64.0                  # host-side weight pre-scale

_CACHE = {}


def _build_nc(repeat=1, skip=(), bufs=None, pipe=3, p2off=4):
    nc = bacc.Bacc("TRN2", target_bir_lowering=False, debug=False)
    dt = mybir.dt
    DR = mybir.MatmulPerfMode.DoubleRow

    a8_d = nc.dram_tensor("a8", [D, NSH], dt.float8e4, kind="ExternalInput")
    b8_d = nc.dram_tensor("b8", [D, NSH], dt.float8e4, kind="ExternalInput")
    w1a_d = nc.dram_tensor("w1a", [D, A], dt.float8e4, kind="ExternalInput")
    w1b_d = nc.dram_tensor("w1b", [D, A], dt.float8e4, kind="ExternalInput")
    w2_d = nc.dram_tensor("w2", [A, H], dt.bfloat16, kind="ExternalInput")
    wca_d = nc.dram_tensor("wca", [D, CP], dt.float8e4, kind="ExternalInput")
    wcb_d = nc.dram_tensor("wcb", [D, CP], dt.float8e4, kind="ExternalInput")
    ones_d = nc.dram_tensor("ones53", [36, CP], dt.bfloat16, kind="ExternalInput")
    id_d = nc.dram_tensor("id53", [128, 128], dt.float32, kind="ExternalInput")
    bc_d = nc.dram_tensor("bc53", [128, 1], dt.float32, kind="ExternalInput")
    out_d = nc.dram_tensor("logits", [BSH, C], dt.float32, kind="ExternalOutput")

    a8_r = a8_d[:].rearrange("(dc p) n -> p dc n", p=128)   # [128,4,NSH]
    b8_r = b8_d[:].rearrange("(dc p) n -> p dc n", p=128)
    out_r = out_d[:].rearrange("(t p) c -> p t c", p=128)   # [128,8,53]

    B = dict(xin=11, th=7, small=7, s=2, z=1, y=2, w=2, t=1)
    if bufs:
        B.update(bufs)
    with tile.TileContext(nc) as tc:
        with (
            tc.tile_pool(name="consts", bufs=1) as consts,
            tc.tile_pool(name="xin", bufs=B["xin"]) as xin,
            tc.tile_pool(name="thp", bufs=B["th"]) as thp,
            tc.tile_pool(name="small", bufs=B["small"]) as small,
            tc.tile_pool(name="persist", bufs=1) as persist,
            tc.tile_pool(name="ps_s", bufs=B["s"], space="PSUM") as ps_s_pool,
            tc.tile_pool(name="ps_z", bufs=B["z"], space="PSUM") as ps_z_pool,
            tc.tile_pool(name="ps_y", bufs=B["y"], space="PSUM") as ps_y_pool,
            tc.tile_pool(name="ps_w", bufs=B["w"], space="PSUM") as ps_w_pool,
            tc.tile_pool(name="ps_t", bufs=B["t"], space="PSUM") as ps_t_pool,
        ):
            # ---- constants via gpsimd SWDGE (keeps HWDGE free for x) ----
            w1a_sb = consts.tile([128, 4, A], dt.float8e4)
            nc.gpsimd.dma_start(out=w1a_sb, in_=w1a_d[:].rearrange("(dc p) a -> p dc a", p=128))
            w1b_sb = consts.tile([128, 4, A], dt.float8e4)
            nc.gpsimd.dma_start(out=w1b_sb, in_=w1b_d[:].rearrange("(dc p) a -> p dc a", p=128))
            w2_sb = consts.tile([128, 2, H], dt.bfloat16)
            nc.gpsimd.dma_start(out=w2_sb, in_=w2_d[:].rearrange("(ah p) h -> p ah h", p=128))
            wca_sb = consts.tile([128, 4, CP], dt.float8e4)
            nc.gpsimd.dma_start(out=wca_sb, in_=wca_d[:].rearrange("(dc p) c -> p dc c", p=128))
            wcb_sb = consts.tile([128, 4, CP], dt.float8e4)
            nc.gpsimd.dma_start(out=wcb_sb, in_=wcb_d[:].rearrange("(dc p) c -> p dc c", p=128))
            ones_sb = consts.tile([36, CP], dt.bfloat16)
            nc.gpsimd.dma_start(out=ones_sb, in_=ones_d[:])
            id_sb = consts.tile([128, 128], dt.float32)
            nc.gpsimd.dma_start(out=id_sb, in_=id_d[:])
            bc_sb = consts.tile([128, 1], dt.float32)
            nc.gpsimd.dma_start(out=bc_sb, in_=bc_d[:])

            for _rep in range(repeat):
              logT = persist.tile([CP, BSH], dt.float32, tag="logT")
              log_out = persist.tile([128, BSH // 128, CP], dt.float32, tag="lo")
              state = {}
              state2 = {}
              state3 = {}
              pstate_z = {}
              pstate_y = {}
              pstate_e = {}
              pstate_t = {}
              pstate_b = {}

              def emit_front(j):
                  a8_t = xin.tile([128, 4, NT], dt.float8e4, tag="a8")
                  b8_t = xin.tile([128, 4, NT], dt.float8e4, tag="b8")
                  if "xdma" not in skip:
                      nc.sync.dma_start(out=a8_t, in_=a8_r[:, :, j * NT:(j + 1) * NT])
                      nc.gpsimd.dma_start(out=b8_t, in_=b8_r[:, :, j * NT:(j + 1) * NT])
                  state[j] = (a8_t, b8_t)

              def emit_mid(j):
                  a8_t, b8_t = state.pop(j)
                  # sT = (a8+b8)@W1s8 + a8@W1sr8, 6 fp8-DR matmuls per a-half
                  th_t = thp.tile([128, 2, NT], dt.bfloat16, tag="th")
                  terms = [(a8_t, w1a_sb), (b8_t, w1a_sb), (a8_t, w1b_sb)]
                  for ah in range(2):
                      ps_s = ps_s_pool.tile([128, NT], dt.float32, tag="s")
                      if "s1" not in skip:
                          for t, (xt, wt) in enumerate(terms):
                              for c in range(2):
                                  nc.tensor.matmul(
                                      ps_s,
                                      wt[:, 2 * c:2 * c + 2, ah * 128:(ah + 1) * 128],
                                      xt[:, 2 * c:2 * c + 2, :],
                                      start=(t == 0 and c == 0),
                                      stop=(t == 2 and c == 1),
                                      perf_mode=DR,
                                  )
                      if "tanh" not in skip:
                          nc.scalar.activation(
                              th_t[:, ah, :], ps_s,
                              mybir.ActivationFunctionType.Tanh, scale=1.0 / WS,
                          )
                  state2[j] = (a8_t, b8_t, th_t)

              # Pair packing: two supertiles share each PSUM tile and each
              # tail engine-op, so the narrow-partition softmax/y ops run at
              # half the per-supertile cost.  z pairs live at partition
              # offsets 0/32 (col_size 32 for a 4-row output), y/wB pairs at
              # 0/64.  Junk partitions between the z slices cost nothing
              # (engines are partition-parallel; time = free size).
              def emit_zst(j):
                  a8_t, b8_t, th_t = state2.pop(j)
                  par = j % 2
                  if par == 0:
                      ps_z2 = ps_z_pool.tile([36, NT], dt.float32, tag="zz")
                      pstate_z[j // 2] = ps_z2
                  else:
                      ps_z2 = pstate_z[j // 2]
                  zs = ps_z2[32 * par:32 * par + H, :]
                  if "z" not in skip:
                      for ah in range(2):
                          nc.tensor.matmul(
                              zs, w2_sb[:, ah, :], th_t[:, ah, :],
                              start=ah == 0, stop=ah == 1,
                          )
                  state3[j] = (a8_t, b8_t)
                  if par == 0:
                      return
                  # pair complete: one exp/den/rden/emul for both supertiles
                  e2 = small.tile([36, NT], dt.bfloat16, tag="e")
                  if "exp" not in skip:
                      nc.scalar.activation(e2, ps_z2,
                                           mybir.ActivationFunctionType.Exp)
                  em2 = small.tile([36, NT], dt.bfloat16, tag="em")
                  if "sm" not in skip:
                      den = small.tile([36, BAGS_PER_SUP], dt.float32, tag="den")
                      nc.vector.tensor_reduce(
                          den, e2.rearrange("h (b i) -> h b i", i=BAG),
                          axis=mybir.AxisListType.X, op=mybir.AluOpType.add,
                      )
                      rdbf = small.tile([36, BAGS_PER_SUP], dt.bfloat16, tag="rd")
                      with nc.allow_low_precision("rden feeds bf16 emul anyway"):
                          nc.vector.reciprocal(rdbf, den)
                      nc.vector.tensor_tensor(
                          out=em2.rearrange("h (b i) -> h b i", i=BAG),
                          in0=e2.rearrange("h (b i) -> h b i", i=BAG),
                          in1=rdbf[:, :, None].broadcast_to(
                              [36, BAGS_PER_SUP, BAG]),
                          op=mybir.AluOpType.mult,
                      )
                  pstate_e[j // 2] = em2

              def emit_yst(j):
                  a8_t, b8_t = state3.pop(j)
                  par = j % 2
                  em2 = pstate_e.pop(j // 2) if par == 1 else pstate_e[j // 2]
                  # wB[c, n] = (0.25/64) * sum_h emul[h, n]  (head-sum + bcast)
                  ps_w = ps_w_pool.tile([CP, NT], dt.float32, tag="w")
                  if "w" not in skip:
                      nc.tensor.matmul(
                          ps_w,
                          ones_sb[32 * par:32 * par + H, :],
                          em2[32 * par:32 * par + H, :],
                          start=True, stop=True,
                      )
                  wb_sb = small.tile([CP, NT], dt.bfloat16, tag="wb")
                  if "wbc" not in skip:
                      nc.scalar.activation(wb_sb, ps_w,
                                           mybir.ActivationFunctionType.Copy)

                  # yT = Wcs8.T(a8+b8) + Wcsr8.T a8, 6 fp8-DR matmuls
                  ps_y = ps_y_pool.tile([CP, NT], dt.float32, tag="y")
                  if "y" not in skip:
                      terms = [(a8_t, wca_sb), (b8_t, wca_sb), (a8_t, wcb_sb)]
                      for t, (xt, wt) in enumerate(terms):
                          for c in range(2):
                              nc.tensor.matmul(
                                  ps_y,
                                  wt[:, 2 * c:2 * c + 2, :],
                                  xt[:, 2 * c:2 * c + 2, :],
                                  start=(t == 0 and c == 0),
                                  stop=(t == 2 and c == 1),
                                  perf_mode=DR,
                              )

                  yw = small.tile([CP, NT], dt.bfloat16, tag="yw")
                  if "yw" not in skip:
                      nc.vector.tensor_tensor(
                          out=yw, in0=ps_y, in1=wb_sb, op=mybir.AluOpType.mult,
                      )
                  if "red" not in skip:
                      nc.vector.tensor_reduce(
                          logT[:, j * BAGS_PER_SUP:(j + 1) * BAGS_PER_SUP],
                          yw.rearrange("c (b i) -> c b i", i=BAG),
                          axis=mybir.AxisListType.X, op=mybir.AluOpType.add,
                      )

              def emit_tail(j):
                  # per 128-bag group: bias, transpose, store
                  if j % 4 != 3 or "tail" in skip:
                      return
                  g = j // 4
                  tmpb = small.tile([CP, 128], dt.float32, tag="tb")
                  nc.vector.tensor_scalar(
                      out=tmpb, in0=logT[:, g * 128:(g + 1) * 128],
                      scalar1=bc_sb[0:CP, :], scalar2=None,
                      op0=mybir.AluOpType.add,
                  )
                  ps_t = ps_t_pool.tile([128, CP], dt.float32, tag="t")
                  nc.tensor.matmul(ps_t, tmpb, id_sb[0:CP, 0:CP], start=True,
                                   stop=True, is_transpose=True)
                  nc.any.tensor_copy(log_out[:, g, :], ps_t)
                  oeng = nc.sync if g == BSH // 128 - 1 else nc.gpsimd
                  oeng.dma_start(out=out_r[:, g, :], in_=log_out[:, g, 0:C])

              P1 = pipe
              P2 = P1 + p2off
              P3 = P2 + 2
              P4 = P3 + 2
              for j in range(NSUP + P4):
                  if j < NSUP:
                      emit_front(j)
                  if P1 <= j < NSUP + P1:
                      emit_mid(j - P1)
                  if P2 <= j < NSUP + P2:
                      emit_zst(j - P2)
                  if P3 <= j < NSUP + P3:
                      emit_yst(j - P3)
                  if j >= P4:
                      emit_tail(j - P4)

    nc.compile()
    return nc


def _get_nc():
    if "nc" not in _CACHE:
        _CACHE["nc"] = _build_nc()
    return _CACHE["nc"]


def _host_prep(x, W1, W2, Wc, bc, seg_ids, n_bags):
    x = np.asarray(x, dtype=np.float32)
    W1 = np.asarray(W1, dtype=np.float32)
    W2 = np.asarray(W2, dtype=np.float32)
    Wc = np.asarray(Wc, dtype=np.float32)
    bc = np.asarray(bc, dtype=np.float32)

    w1s = WS * W1
    w1a = w1s.astype(F8)
    w1b = (w1s - w1a.astype(np.float32)).astype(F8)
    wc_pad = np.zeros((D, CP), np.float32)
    wc_pad[:, :C] = Wc
    wcs = WS * wc_pad
    wca = wcs.astype(F8)
    wcb = (wcs - wca.astype(np.float32)).astype(F8)
    w2b = W2.astype(BF16)
    ones53 = np.full((36, CP), 0.25 / WS, np.float32).astype(BF16)
    id53 = np.eye(128, dtype=np.float32)
    bc53 = np.zeros((CP, 1), np.float32)
    bc53[:C, 0] = bc
    bc53 = np.tile(bc53, (2, 1))

    in_maps = []
    for c in range(NCORES):
        xT = np.ascontiguousarray(x[c * NSH:(c + 1) * NSH].T)
        a8 = xT.astype(F8)
        b8 = (xT - a8.astype(np.float32)).astype(F8)
        in_maps.append({
            "a8": a8, "b8": b8,
            "w1a": w1a, "w1b": w1b, "w2": w2b,
            "wca": wca, "wcb": wcb,
            "ones53": ones53, "id53": id53, "bc53": bc53,
        })
    return in_maps


def kernel(x, W1, W2, Wc, bc, seg_ids, n_bags, _trace=False):
    in_maps = _host_prep(x, W1, W2, Wc, bc, seg_ids, n_bags)
    nc = _get_nc()
    res = bass_utils.run_bass_kernel_spmd(
        nc, in_maps, core_ids=list(range(NCORES)), trace=_trace,
    )
    out = np.concatenate([r["logits"] for r in res.results], axis=0)
    if _trace:
        kernel.last_results = res
    return out
